# revision 46
# baseline (speedup 1.0000x reference)
"""Bass/Trainium2 kernel for nn_BayesianResNet_71408126263673.

Grouped per-sample conv: for each of 32 samples i,
  out[i] = conv2d(x[i] [128,32,32], W[i] [128oc,128c,3,3], pad=1, stride=1) + bias[i]

Sharding: b_i (32 samples) split across 8 NeuronCores, 4 samples per core.
Pure data parallel, no collectives.

Per-core kernel: each sample's conv is computed as 9 accumulating matmuls
(one per 3x3 tap) into PSUM:
  out[oc, pix] = sum_{kh,kw} W[:, :, kh, kw].T @ xpad[:, shifted pix]
with K=c=128 (partition/contraction), M=oc=128, N=512 pixels (16 output rows
per PSUM bank). The input image is zero-padded to 34x34 on the HOST so DMA
loads are fully contiguous and no memset/masking is needed on-chip. Weights
are pre-transposed on the host to [c, kh*kw, oc] so each tap is a ready-to-use
lhsT (stationary operand) tile.
"""

import os
import numpy as np

import concourse.bacc as bacc
import concourse.tile as tile
from concourse import mybir
from concourse.bass_utils import run_bass_kernel_spmd

N_CORES = 8
B_I, B_J, C, H, W = 32, 1, 128, 32, 32
OC, KH, KW = 128, 3, 3
S = B_I // N_CORES            # samples per core
HP, WP = H + 2, W + 2         # padded image
NTAP = KH * KW                # 9
NBLK = 2                      # output row blocks per sample
RPB = H // NBLK               # 16 rows per block -> N = 512

_DT_TABLE = {
    "fp32": (mybir.dt.float32, np.float32),
    "fp32r": (mybir.dt.float32r, np.float32),
    "fp16": (mybir.dt.float16, np.float16),
    "bf16": (mybir.dt.bfloat16, None),  # np dtype filled lazily below
}

# Matmul operand dtype (walrus requires x and w to be both 16-bit or both
# 32-bit). Default fp16: 1 PE cycle/row with fast weight load, measured rel
# err ~2.9e-4 vs the fp32 reference. fp32r gives ~1.5e-4 at ~+15% time;
# fp32 gives ~3e-7 at ~2.5x time.
_MM_DT_NAME = os.environ.get("CONV_MM_DTYPE", "fp16")
MM_DT, MM_NP = _DT_TABLE[_MM_DT_NAME]
if MM_NP is None:
    import ml_dtypes

    MM_NP = ml_dtypes.bfloat16
X_DT = W_DT = MM_DT
X_NP = W_NP = MM_NP

# test.py hooks: set TRACE=True before calling kernel() to profile; the
# BassKernelResults of the last run lands in LAST_RESULTS.
TRACE = False
TRACE_KW = {}
LAST_RESULTS = None

_NC_CACHE = None


def _build_nc():
    f32 = mybir.dt.float32
    nc = bacc.Bacc()
    # Weights and padded image are concatenated per sample on the host into
    # one [C, 1152+1156] buffer: one DMA, one dispatch, one completion
    # semaphore per sample, with full-width per-partition chunks.
    xw_d = nc.declare_dram_parameter(
        "xw", [S, C, NTAP * OC + HP * WP], MM_DT, isOutput=False
    )
    b_d = nc.declare_dram_parameter("b", [OC, S], f32, isOutput=False)
    o_d = nc.declare_dram_parameter("o", [S, OC, H, W], f32, isOutput=True)
    # The very last row-block is emitted in fp16 (half the bytes on the
    # kernel's critical tail); the host casts it back to f32.
    o2_d = nc.declare_dram_parameter("o2", [OC, RPB, W], mybir.dt.float16, isOutput=True)

    with tile.TileContext(nc) as tc:
        with (
            tc.tile_pool(name="ins", bufs=1) as ins_pool,
            tc.tile_pool(name="outs", bufs=1) as outs_pool,
            tc.tile_pool(name="psum", bufs=8, space="PSUM") as psum_pool,
        ):
            # PE warmup: ~32 dependency-free matmuls on garbage data keep the
            # PE busy from engine start so the HAM clock-gate reaches 2.4 GHz
            # before the first real matmul (otherwise the first ~3.4us of
            # matmuls run at 1.2 GHz). Their PSUM tile is never read.
            wu_x = ins_pool.tile([C, OC], W_DT, tag="warmup", name="warmup")
            nc.gpsimd.memset(wu_x[:], 0.0)
            wu_ps = psum_pool.tile([C, OC], f32, name="wu_ps", tag="ps")
            for _ in range(32):
                nc.tensor.matmul(wu_ps[:], wu_x[:], wu_x[:], start=True, stop=True)

            xw_ts = [
                ins_pool.tile(
                    [C, NTAP * OC + HP * WP], MM_DT, tag=f"xw{s}", name=f"xw{s}"
                )
                for s in range(S)
            ]
            wts = [t[:, : NTAP * OC] for t in xw_ts]
            xvs = [
                t[:, NTAP * OC :].rearrange("p (h w) -> p h w", w=WP)
                for t in xw_ts
            ]
            bias_t = ins_pool.tile([OC, S], f32, tag="bias")

            # The SP HWDGE queue ramps to full rate immediately while the ACT
            # queue starts slow, so samples 0/1 go on SP in deadline order;
            # samples 2/3 + the bias ride the ACT queue, whose slow ramp
            # doesn't matter (first needed ~10us later).
            nc.sync.dma_start(xw_ts[0][:], xw_d[0])
            nc.scalar.dma_start(bias_t[:], b_d[:])
            nc.sync.dma_start(xw_ts[1][:], xw_d[1])
            nc.scalar.dma_start(xw_ts[2][:], xw_d[2])
            nc.scalar.dma_start(xw_ts[3][:], xw_d[3])

            def conv_block(s, row0, nrows, ps_name):
                """One accumulation group: output rows [row0, row0+nrows)."""
                ps = psum_pool.tile([OC, nrows, W], f32, name=ps_name, tag="ps")
                for t in range(NTAP):
                    kh, kw = divmod(t, KW)
                    rhs = xvs[s][:, row0 + kh : row0 + kh + nrows, kw : kw + W]
                    lhsT = wts[s][:, t * OC : (t + 1) * OC]
                    nc.tensor.matmul(
                        ps[:], lhsT, rhs, start=(t == 0), stop=(t == NTAP - 1)
                    )
                return ps

            for s in range(S):
                out_t = outs_pool.tile([OC, H, W], f32, tag=f"out{s}", name=f"out{s}")
                blocks = [(0, RPB), (RPB, RPB)]
                for bi, (row0, nrows) in enumerate(blocks):
                    ps = conv_block(s, row0, nrows, f"ps{s}_{bi}")
                    if s == S - 1 and bi == NBLK - 1:
                        out16_t = outs_pool.tile(
                            [OC, RPB, W], mybir.dt.float16, tag="out16", name="out16"
                        )
                        nc.scalar.activation(
                            out16_t[:],
                            ps[:],
                            mybir.ActivationFunctionType.Identity,
                            bias=bias_t[:, s : s + 1],
                        )
                        nc.sync.dma_start(o2_d[:], out16_t[:])
                        continue
                    nc.scalar.activation(
                        out_t[:, row0 : row0 + nrows, :],
                        ps[:],
                        mybir.ActivationFunctionType.Identity,
                        bias=bias_t[:, s : s + 1],
                    )
                    if s == S - 1:
                        nc.scalar.dma_start(
                            o_d[s][:, row0 : row0 + nrows, :],
                            out_t[:, row0 : row0 + nrows, :],
                        )
                if s < S - 1:
                    nc.sync.dma_start(o_d[s], out_t[:])
    nc.compile()
    return nc


def _get_nc():
    global _NC_CACHE
    if _NC_CACHE is None:
        _NC_CACHE = _build_nc()
    return _NC_CACHE


def kernel(x: np.ndarray, weight: np.ndarray, bias: np.ndarray) -> np.ndarray:
    global LAST_RESULTS
    assert x.shape == (B_I, B_J, C, H, W)
    assert weight.shape == (B_I, OC, C, KH, KW)
    assert bias.shape == (B_I, B_J, OC)

    x = np.asarray(x, dtype=np.float32)
    weight = np.asarray(weight, dtype=np.float32)
    bias = np.asarray(bias, dtype=np.float32)

    # Host-side layout prep (part of sharding): zero-pad images, transpose
    # weights so each 3x3 tap is a contiguous [c, oc] stationary tile.
    xw = np.zeros((B_I, C, NTAP * OC + HP * WP), dtype=MM_NP)
    wt = np.ascontiguousarray(weight.transpose(0, 2, 3, 4, 1))  # [b_i, c, kh, kw, oc]
    xw[:, :, : NTAP * OC] = wt.reshape(B_I, C, NTAP * OC).astype(MM_NP)
    xpad = xw[:, :, NTAP * OC :].reshape(B_I, C, HP, WP)
    xpad[:, :, 1 : 1 + H, 1 : 1 + W] = x[:, 0].astype(MM_NP)
    bt = bias[:, 0, :]  # [b_i, oc]

    in_maps = []
    for core in range(N_CORES):
        sl = slice(core * S, (core + 1) * S)
        in_maps.append(
            {
                "xw": np.ascontiguousarray(xw[sl]),
                "b": np.ascontiguousarray(bt[sl].T),  # [OC, S]
            }
        )

    nc = _get_nc()
    res = run_bass_kernel_spmd(
        nc, in_maps, core_ids=list(range(N_CORES)), trace=TRACE, **TRACE_KW
    )
    LAST_RESULTS = res

    outs = []
    for c in range(N_CORES):
        o = np.array(res.results[c]["o"])
        o[S - 1, :, RPB:, :] = res.results[c]["o2"].astype(np.float32)
        outs.append(o)
    out = np.concatenate(outs, axis=0)
    return out.reshape(B_I, B_J, OC, H, W)


# revision 47
# speedup vs baseline: 1.0195x; 1.0195x over previous
"""Bass/Trainium2 kernel for nn_BayesianResNet_71408126263673.

Grouped per-sample conv: for each of 32 samples i,
  out[i] = conv2d(x[i] [128,32,32], W[i] [128oc,128c,3,3], pad=1, stride=1) + bias[i]

Sharding: b_i (32 samples) split across 8 NeuronCores, 4 samples per core.
Pure data parallel, no collectives.

Per-core kernel: each sample's conv is computed as 9 accumulating matmuls
(one per 3x3 tap) into PSUM:
  out[oc, pix] = sum_{kh,kw} W[:, :, kh, kw].T @ xpad[:, shifted pix]
with K=c=128 (partition/contraction), M=oc=128, N=512 pixels (16 output rows
per PSUM bank). The input image is zero-padded to 34x34 on the HOST so DMA
loads are fully contiguous and no memset/masking is needed on-chip. Weights
are pre-transposed on the host to [c, kh*kw, oc] so each tap is a ready-to-use
lhsT (stationary operand) tile.
"""

import os
import numpy as np

import concourse.bacc as bacc
import concourse.tile as tile
from concourse import mybir
from concourse.bass_utils import run_bass_kernel_spmd

N_CORES = 8
B_I, B_J, C, H, W = 32, 1, 128, 32, 32
OC, KH, KW = 128, 3, 3
S = B_I // N_CORES            # samples per core
HP, WP = H + 2, W + 2         # padded image
NTAP = KH * KW                # 9
NBLK = 2                      # output row blocks per sample
RPB = H // NBLK               # 16 rows per block -> N = 512

_DT_TABLE = {
    "fp32": (mybir.dt.float32, np.float32),
    "fp32r": (mybir.dt.float32r, np.float32),
    "fp16": (mybir.dt.float16, np.float16),
    "bf16": (mybir.dt.bfloat16, None),  # np dtype filled lazily below
}

# Matmul operand dtype (walrus requires x and w to be both 16-bit or both
# 32-bit). Default fp16: 1 PE cycle/row with fast weight load, measured rel
# err ~2.9e-4 vs the fp32 reference. fp32r gives ~1.5e-4 at ~+15% time;
# fp32 gives ~3e-7 at ~2.5x time.
_MM_DT_NAME = os.environ.get("CONV_MM_DTYPE", "fp16")
MM_DT, MM_NP = _DT_TABLE[_MM_DT_NAME]
if MM_NP is None:
    import ml_dtypes

    MM_NP = ml_dtypes.bfloat16
X_DT = W_DT = MM_DT
X_NP = W_NP = MM_NP

# test.py hooks: set TRACE=True before calling kernel() to profile; the
# BassKernelResults of the last run lands in LAST_RESULTS.
TRACE = False
TRACE_KW = {}
LAST_RESULTS = None

_NC_CACHE = None


def _build_nc():
    f32 = mybir.dt.float32
    nc = bacc.Bacc()
    # Weights and padded image are concatenated per sample on the host into
    # one [C, 1152+1156] buffer: one DMA, one dispatch, one completion
    # semaphore per sample, with full-width per-partition chunks.
    xw_d = nc.declare_dram_parameter(
        "xw", [S, C, NTAP * OC + HP * WP], MM_DT, isOutput=False
    )
    b_d = nc.declare_dram_parameter("b", [OC, S], f32, isOutput=False)
    o_d = nc.declare_dram_parameter("o", [S, OC, H, W], f32, isOutput=True)

    with tile.TileContext(nc) as tc:
        with (
            tc.tile_pool(name="ins", bufs=1) as ins_pool,
            tc.tile_pool(name="outs", bufs=1) as outs_pool,
            tc.tile_pool(name="psum", bufs=8, space="PSUM") as psum_pool,
        ):
            # PE warmup: ~32 dependency-free matmuls on garbage data keep the
            # PE busy from engine start so the HAM clock-gate reaches 2.4 GHz
            # before the first real matmul (otherwise the first ~3.4us of
            # matmuls run at 1.2 GHz). Their PSUM tile is never read.
            wu_x = ins_pool.tile([C, OC], W_DT, tag="warmup", name="warmup")
            nc.gpsimd.memset(wu_x[:], 0.0)
            wu_ps = psum_pool.tile([C, OC], f32, name="wu_ps", tag="ps")
            for _ in range(32):
                nc.tensor.matmul(wu_ps[:], wu_x[:], wu_x[:], start=True, stop=True)

            xw_ts = [
                ins_pool.tile(
                    [C, NTAP * OC + HP * WP], MM_DT, tag=f"xw{s}", name=f"xw{s}"
                )
                for s in range(S)
            ]
            wts = [t[:, : NTAP * OC] for t in xw_ts]
            xvs = [
                t[:, NTAP * OC :].rearrange("p (h w) -> p h w", w=WP)
                for t in xw_ts
            ]
            bias_t = ins_pool.tile([OC, S], f32, tag="bias")

            # The SP HWDGE queue ramps to full rate immediately while the ACT
            # queue starts slow, so samples 0/1 go on SP in deadline order;
            # samples 2/3 + the bias ride the ACT queue, whose slow ramp
            # doesn't matter (first needed ~10us later).
            nc.sync.dma_start(xw_ts[0][:], xw_d[0])
            nc.scalar.dma_start(bias_t[:], b_d[:])
            nc.sync.dma_start(xw_ts[1][:], xw_d[1])
            nc.scalar.dma_start(xw_ts[2][:], xw_d[2])
            nc.scalar.dma_start(xw_ts[3][:], xw_d[3])

            def conv_block(s, row0, nrows, ps_name):
                """One accumulation group: output rows [row0, row0+nrows)."""
                ps = psum_pool.tile([OC, nrows, W], f32, name=ps_name, tag="ps")
                for t in range(NTAP):
                    kh, kw = divmod(t, KW)
                    rhs = xvs[s][:, row0 + kh : row0 + kh + nrows, kw : kw + W]
                    lhsT = wts[s][:, t * OC : (t + 1) * OC]
                    nc.tensor.matmul(
                        ps[:], lhsT, rhs, start=(t == 0), stop=(t == NTAP - 1)
                    )
                return ps

            for s in range(S):
                out_t = outs_pool.tile([OC, H, W], f32, tag=f"out{s}", name=f"out{s}")
                blocks = [(0, RPB), (RPB, RPB)]
                for bi, (row0, nrows) in enumerate(blocks):
                    ps = conv_block(s, row0, nrows, f"ps{s}_{bi}")
                    nc.scalar.activation(
                        out_t[:, row0 : row0 + nrows, :],
                        ps[:],
                        mybir.ActivationFunctionType.Identity,
                        bias=bias_t[:, s : s + 1],
                    )
                    if s == S - 1:
                        # Split the last sample's store per row-block on both
                        # queues so the two 256KB halves stream in parallel.
                        eng = nc.scalar if bi == 0 else nc.sync
                        eng.dma_start(
                            o_d[s][:, row0 : row0 + nrows, :],
                            out_t[:, row0 : row0 + nrows, :],
                        )
                if s < S - 1:
                    nc.sync.dma_start(o_d[s], out_t[:])
    nc.compile()
    return nc


def _get_nc():
    global _NC_CACHE
    if _NC_CACHE is None:
        _NC_CACHE = _build_nc()
    return _NC_CACHE


def kernel(x: np.ndarray, weight: np.ndarray, bias: np.ndarray) -> np.ndarray:
    global LAST_RESULTS
    assert x.shape == (B_I, B_J, C, H, W)
    assert weight.shape == (B_I, OC, C, KH, KW)
    assert bias.shape == (B_I, B_J, OC)

    x = np.asarray(x, dtype=np.float32)
    weight = np.asarray(weight, dtype=np.float32)
    bias = np.asarray(bias, dtype=np.float32)

    # Host-side layout prep (part of sharding): zero-pad images, transpose
    # weights so each 3x3 tap is a contiguous [c, oc] stationary tile.
    xw = np.zeros((B_I, C, NTAP * OC + HP * WP), dtype=MM_NP)
    wt = np.ascontiguousarray(weight.transpose(0, 2, 3, 4, 1))  # [b_i, c, kh, kw, oc]
    xw[:, :, : NTAP * OC] = wt.reshape(B_I, C, NTAP * OC).astype(MM_NP)
    xpad = xw[:, :, NTAP * OC :].reshape(B_I, C, HP, WP)
    xpad[:, :, 1 : 1 + H, 1 : 1 + W] = x[:, 0].astype(MM_NP)
    bt = bias[:, 0, :]  # [b_i, oc]

    in_maps = []
    for core in range(N_CORES):
        sl = slice(core * S, (core + 1) * S)
        in_maps.append(
            {
                "xw": np.ascontiguousarray(xw[sl]),
                "b": np.ascontiguousarray(bt[sl].T),  # [OC, S]
            }
        )

    nc = _get_nc()
    res = run_bass_kernel_spmd(
        nc, in_maps, core_ids=list(range(N_CORES)), trace=TRACE, **TRACE_KW
    )
    LAST_RESULTS = res

    out = np.concatenate([res.results[c]["o"] for c in range(N_CORES)], axis=0)
    return out.reshape(B_I, B_J, OC, H, W)


# revision 48
# speedup vs baseline: 1.0449x; 1.0249x over previous
"""Bass/Trainium2 kernel for nn_BayesianResNet_71408126263673.

Grouped per-sample conv: for each of 32 samples i,
  out[i] = conv2d(x[i] [128,32,32], W[i] [128oc,128c,3,3], pad=1, stride=1) + bias[i]

Sharding: b_i (32 samples) split across 8 NeuronCores, 4 samples per core.
Pure data parallel, no collectives.

Per-core kernel: each sample's conv is computed as 9 accumulating matmuls
(one per 3x3 tap) into PSUM:
  out[oc, pix] = sum_{kh,kw} W[:, :, kh, kw].T @ xpad[:, shifted pix]
with K=c=128 (partition/contraction), M=oc=128, N=512 pixels (16 output rows
per PSUM bank). The input image is zero-padded to 34x34 on the HOST so DMA
loads are fully contiguous and no memset/masking is needed on-chip. Weights
are pre-transposed on the host to [c, kh*kw, oc] so each tap is a ready-to-use
lhsT (stationary operand) tile.
"""

import os
import numpy as np

import concourse.bacc as bacc
import concourse.tile as tile
from concourse import mybir
from concourse.bass_utils import run_bass_kernel_spmd

N_CORES = 8
B_I, B_J, C, H, W = 32, 1, 128, 32, 32
OC, KH, KW = 128, 3, 3
S = B_I // N_CORES            # samples per core
HP, WP = H + 2, W + 2         # padded image
NTAP = KH * KW                # 9
NBLK = 2                      # output row blocks per sample
RPB = H // NBLK               # 16 rows per block -> N = 512

_DT_TABLE = {
    "fp32": (mybir.dt.float32, np.float32),
    "fp32r": (mybir.dt.float32r, np.float32),
    "fp16": (mybir.dt.float16, np.float16),
    "bf16": (mybir.dt.bfloat16, None),  # np dtype filled lazily below
}

# Matmul operand dtype (walrus requires x and w to be both 16-bit or both
# 32-bit). Default fp16: 1 PE cycle/row with fast weight load, measured rel
# err ~2.9e-4 vs the fp32 reference. fp32r gives ~1.5e-4 at ~+15% time;
# fp32 gives ~3e-7 at ~2.5x time.
_MM_DT_NAME = os.environ.get("CONV_MM_DTYPE", "fp16")
MM_DT, MM_NP = _DT_TABLE[_MM_DT_NAME]
if MM_NP is None:
    import ml_dtypes

    MM_NP = ml_dtypes.bfloat16
X_DT = W_DT = MM_DT
X_NP = W_NP = MM_NP

# test.py hooks: set TRACE=True before calling kernel() to profile; the
# BassKernelResults of the last run lands in LAST_RESULTS.
TRACE = False
TRACE_KW = {}
LAST_RESULTS = None

_NC_CACHE = None


def _build_nc():
    f32 = mybir.dt.float32
    nc = bacc.Bacc()
    # Weights and padded image are concatenated per sample on the host into
    # one [C, 1152+1156] buffer: one DMA, one dispatch, one completion
    # semaphore per sample, with full-width per-partition chunks.
    xw_d = nc.declare_dram_parameter(
        "xw", [S, C, NTAP * OC + HP * WP], MM_DT, isOutput=False
    )
    b_d = nc.declare_dram_parameter("b", [OC, S], f32, isOutput=False)
    o_d = nc.declare_dram_parameter("o", [S, OC, H, W], f32, isOutput=True)

    with tile.TileContext(nc) as tc:
        with (
            tc.tile_pool(name="ins", bufs=1) as ins_pool,
            tc.tile_pool(name="outs", bufs=1) as outs_pool,
            tc.tile_pool(name="psum", bufs=8, space="PSUM") as psum_pool,
        ):
            # PE warmup: ~32 dependency-free matmuls on garbage data keep the
            # PE busy from engine start so the HAM clock-gate reaches 2.4 GHz
            # before the first real matmul (otherwise the first ~3.4us of
            # matmuls run at 1.2 GHz). Their PSUM tile is never read.
            wu_x = ins_pool.tile([C, OC], W_DT, tag="warmup", name="warmup")
            nc.gpsimd.memset(wu_x[:], 0.0)
            wu_ps = psum_pool.tile([C, OC], f32, name="wu_ps", tag="ps")
            for _ in range(32):
                nc.tensor.matmul(wu_ps[:], wu_x[:], wu_x[:], start=True, stop=True)

            xw_ts = [
                ins_pool.tile(
                    [C, NTAP * OC + HP * WP], MM_DT, tag=f"xw{s}", name=f"xw{s}"
                )
                for s in range(S)
            ]
            wts = [t[:, : NTAP * OC] for t in xw_ts]
            xvs = [
                t[:, NTAP * OC :].rearrange("p (h w) -> p h w", w=WP)
                for t in xw_ts
            ]
            bias_t = ins_pool.tile([OC, S], f32, tag="bias")

            # The SP HWDGE queue ramps to full rate immediately while the ACT
            # queue starts slow, so samples 0/1 go on SP in deadline order;
            # samples 2/3 + the bias ride the ACT queue, whose slow ramp
            # doesn't matter (first needed ~10us later).
            nc.sync.dma_start(xw_ts[0][:], xw_d[0])
            nc.scalar.dma_start(bias_t[:], b_d[:])
            nc.sync.dma_start(xw_ts[1][:], xw_d[1])
            nc.scalar.dma_start(xw_ts[2][:], xw_d[2])
            nc.scalar.dma_start(xw_ts[3][:], xw_d[3])

            def conv_block(s, row0, nrows, ps_name):
                """One accumulation group: output rows [row0, row0+nrows)."""
                ps = psum_pool.tile([OC, nrows, W], f32, name=ps_name, tag="ps")
                for t in range(NTAP):
                    kh, kw = divmod(t, KW)
                    rhs = xvs[s][:, row0 + kh : row0 + kh + nrows, kw : kw + W]
                    lhsT = wts[s][:, t * OC : (t + 1) * OC]
                    nc.tensor.matmul(
                        ps[:], lhsT, rhs, start=(t == 0), stop=(t == NTAP - 1)
                    )
                return ps

            for s in range(S):
                out_t = outs_pool.tile([OC, H, W], f32, tag=f"out{s}", name=f"out{s}")
                blocks = [(0, RPB), (RPB, RPB)]
                for bi, (row0, nrows) in enumerate(blocks):
                    ps = conv_block(s, row0, nrows, f"ps{s}_{bi}")
                    nc.scalar.activation(
                        out_t[:, row0 : row0 + nrows, :],
                        ps[:],
                        mybir.ActivationFunctionType.Identity,
                        bias=bias_t[:, s : s + 1],
                    )
                    if s == S - 1:
                        # Split the last sample's store per row-block on both
                        # queues so the two 256KB halves stream in parallel.
                        eng = nc.scalar if bi == 0 else nc.sync
                        eng.dma_start(
                            o_d[s][:, row0 : row0 + nrows, :],
                            out_t[:, row0 : row0 + nrows, :],
                        )
                if s < S - 1:
                    nc.sync.dma_start(o_d[s], out_t[:])
    nc.compile()
    return nc


def _get_nc():
    global _NC_CACHE
    if _NC_CACHE is None:
        _NC_CACHE = _build_nc()
    return _NC_CACHE


def kernel(x: np.ndarray, weight: np.ndarray, bias: np.ndarray) -> np.ndarray:
    global LAST_RESULTS
    assert x.shape == (B_I, B_J, C, H, W)
    assert weight.shape == (B_I, OC, C, KH, KW)
    assert bias.shape == (B_I, B_J, OC)

    x = np.asarray(x, dtype=np.float32)
    weight = np.asarray(weight, dtype=np.float32)
    bias = np.asarray(bias, dtype=np.float32)

    # Host-side layout prep (part of sharding): zero-pad images, transpose
    # weights so each 3x3 tap is a contiguous [c, oc] stationary tile.
    xw = np.zeros((B_I, C, NTAP * OC + HP * WP), dtype=MM_NP)
    wt = np.ascontiguousarray(weight.transpose(0, 2, 3, 4, 1))  # [b_i, c, kh, kw, oc]
    xw[:, :, : NTAP * OC] = wt.reshape(B_I, C, NTAP * OC).astype(MM_NP)
    xpad = xw[:, :, NTAP * OC :].reshape(B_I, C, HP, WP)
    xpad[:, :, 1 : 1 + H, 1 : 1 + W] = x[:, 0].astype(MM_NP)
    bt = bias[:, 0, :]  # [b_i, oc]

    in_maps = []
    for core in range(N_CORES):
        sl = slice(core * S, (core + 1) * S)
        in_maps.append(
            {
                "xw": np.ascontiguousarray(xw[sl]),
                "b": np.ascontiguousarray(bt[sl].T),  # [OC, S]
            }
        )

    nc = _get_nc()
    try:
        res = run_bass_kernel_spmd(
            nc, in_maps, core_ids=list(range(N_CORES)), trace=TRACE, **TRACE_KW
        )
    except Exception:
        # Transient NRT/device errors (e.g. NRT_EXEC_UNIT_UNRECOVERABLE after
        # heavy reuse) usually clear on retry; the work is idempotent.
        import time

        time.sleep(10)
        res = run_bass_kernel_spmd(
            nc, in_maps, core_ids=list(range(N_CORES)), trace=TRACE, **TRACE_KW
        )
    LAST_RESULTS = res

    out = np.concatenate([res.results[c]["o"] for c in range(N_CORES)], axis=0)
    return out.reshape(B_I, B_J, OC, H, W)
